# revision 1
# baseline (speedup 1.0000x reference)
"""Chamfer distance L2 kernel for Trainium2 (8 NeuronCores).

Problem: xyz1 [4, 8192, 3] f32, xyz2 [4, 8192, 3] f32.
Outputs: dist1 [4, 8192] (min_j ||xyz1[b,i]-xyz2[b,j]||^2),
         dist2 [4, 8192] (min_i over xyz1 for each xyz2 point).

Sharding: 4 batches x 2 directions = 8 independent jobs, one per core.
Each core: queries q [8192,3], refs r [8192,3] -> dist [8192].

Per-core algorithm:
  d_ij = sq_i + sq_j - 2 q_i . r_j  computed on the PE as a K=24 matmul:
  each fp32 value is split into 3 bf16 terms (h+m+l); the 6 dominant
  cross products per coordinate (hh, hm, mh, hl, lh, mm) plus 3-term
  splits of the two squared norms give fp32-grade accuracy at full bf16
  PE speed (fp32 matmul would be 4x slower). The K-major [24, 8192]
  bf16 augmented layouts are precomputed on the host (cheap O(N) prep),
  so the device runs no prologue beyond two contiguous DMAs.
  Consumers are balanced across two engines: per query tile, chunk 0 of
  the PSUM distance row is min-reduced by the DVE directly from PSUM,
  chunks 1-3 are copied PSUM->SBUF fp16 by the ACT engine while the DVE
  folds them at its 2x 16-bit rate.
"""

import sys

for _p in ("/opt/trn_rl_repo", "/root/.axon_site/_ro/trn_rl_repo"):
    if _p not in sys.path:
        sys.path.insert(0, _p)

import ml_dtypes
import numpy as np

import concourse.bacc as bacc
import concourse.mybir as mybir
from concourse.bass_utils import run_bass_kernel_spmd
from concourse.masks import make_identity
from concourse.tile import TileContext

B = 4
N = 8192          # points per cloud
P = 128           # partitions
NQT = N // P      # 64 query tiles
CHUNK = 2048      # refs per consumer chunk (4 PSUM banks)
NCHUNK = N // CHUNK
MM_N = 512        # matmul moving free dim (1 PSUM bank fp32)
K = 24            # contraction rows after 3-term bf16 split

F32 = mybir.dt.float32
BF16 = mybir.dt.bfloat16
F16 = mybir.dt.float16

BF = ml_dtypes.bfloat16


def _split3_np(x):
    """3-term bf16 split: x ~= h + m + l (all returned as fp32 arrays)."""
    h = x.astype(BF).astype(np.float32)
    r1 = x - h
    m = r1.astype(BF).astype(np.float32)
    r2 = r1 - m
    l = r2.astype(BF).astype(np.float32)
    return h, m, l


def _build_aug_np(pts, is_query):
    """Host-side K-major augmented layout [24, 8192] bf16.

    Row k of the query layout pairs with row k of the ref layout so that
    sum_k q_k * r_k = sq_q + sq_r - 2 q.r  (to ~fp32 accuracy).
    """
    pts = np.asarray(pts, dtype=np.float32)
    sq = (pts * pts).sum(-1)                      # [N]
    base = (-2.0 * pts) if is_query else pts
    ch, cm, cl = _split3_np(base)                 # [N, 3] each
    sh, sm, sl = _split3_np(sq)                   # [N]
    ones = np.ones_like(sq)
    rows = []
    for c in range(3):
        if is_query:
            rows += [ch[:, c], ch[:, c], cm[:, c], ch[:, c], cl[:, c], cm[:, c]]
        else:
            rows += [ch[:, c], cm[:, c], ch[:, c], cl[:, c], ch[:, c], cm[:, c]]
    if is_query:
        rows += [sh, sm, sl, ones, ones, ones]
    else:
        rows += [ones, ones, ones, sh, sm, sl]
    return np.ascontiguousarray(np.stack(rows, 0).astype(BF))


def build_program():
    nc = bacc.Bacc("TRN2", target_bir_lowering=False, debug=False)
    aq_dram = nc.dram_tensor("aq", [K, N], BF16, kind="ExternalInput").ap()
    ar_dram = nc.dram_tensor("ar", [K, N], BF16, kind="ExternalInput").ap()
    out_dram = nc.dram_tensor("dist", [N], F32, kind="ExternalOutput").ap()

    with TileContext(nc) as tc:
        from contextlib import ExitStack
        with ExitStack() as ctx:
            consts = ctx.enter_context(tc.tile_pool(name="consts", bufs=1))
            identity_f32 = consts.tile([P, P], F32)
            make_identity(nc, identity_f32)
            augT_q = consts.tile([K, N], BF16)
            augT_r = consts.tile([K, N], BF16)
            dist_sb = consts.tile([P, NQT], F32)
            # contiguous row-major loads; each partition gets a 16KB stream
            nc.sync.dma_start(out=augT_q, in_=aq_dram)
            nc.sync.dma_start(out=augT_r, in_=ar_dram)

            # ---- main loop ----
            # Per query tile: 4 PSUM chunks of 2048 refs. Chunk 0 is
            # min-reduced by the DVE straight from PSUM (1x). Chunks 1-3 are
            # copied PSUM->SBUF fp16 by the ACT engine (1x, in parallel) and
            # the DVE folds those at its 2x fp16 rate — balancing the two
            # engines instead of serializing everything through the DVE.
            MIN = mybir.AluOpType.min
            X = mybir.AxisListType.X
            H = CHUNK // 2
            with tc.tile_pool(name="mm_psum", bufs=2, space="PSUM") as mm_psum, \
                 tc.tile_pool(name="stage", bufs=3, space="SBUF") as stage_pool, \
                 tc.tile_pool(name="small", bufs=8) as small_pool:
                for qt in range(NQT):
                    lhsT = augT_q[:, qt * P:(qt + 1) * P]
                    stage = stage_pool.tile([P, 3, CHUNK], F16, tag="stage")
                    partA = small_pool.tile([P, 1], F32, tag="partA")
                    for ch in range(NCHUNK):
                        ps = mm_psum.tile([P, CHUNK], F32, tag="ps")
                        for j in range(CHUNK // MM_N):
                            col = ch * CHUNK + j * MM_N
                            nc.tensor.matmul(
                                ps[:, j * MM_N:(j + 1) * MM_N],
                                lhsT,
                                augT_r[:, col:col + MM_N],
                                start=True,
                                stop=True,
                            )
                        if ch == 0:
                            # DVE min-reduces this chunk straight from PSUM
                            nc.vector.tensor_reduce(partA, ps, axis=X, op=MIN)
                        else:
                            # ACT copies to fp16 SBUF for 2x DVE folds
                            nc.scalar.copy(stage[:, ch - 1, :], ps)
                    # fp16 fold tree on DVE (2x mode, SBUF step-1)
                    m1 = stage_pool.tile([P, CHUNK], F16, tag="m1")
                    nc.vector.tensor_tensor(m1, stage[:, 0, :], stage[:, 1, :], op=MIN)
                    f1 = stage_pool.tile([P, H], F16, tag="f1")
                    nc.vector.tensor_tensor(f1, m1[:, :H], m1[:, H:], op=MIN)
                    g1 = stage_pool.tile([P, H], F16, tag="g1")
                    nc.vector.tensor_tensor(
                        g1, stage[:, 2, :H], stage[:, 2, H:], op=MIN)
                    f2 = stage_pool.tile([P, H], F16, tag="f2")
                    nc.vector.tensor_tensor(f2, f1, g1, op=MIN)
                    f3 = stage_pool.tile([P, H // 2], F16, tag="f3")
                    nc.vector.tensor_tensor(f3, f2[:, :H // 2], f2[:, H // 2:], op=MIN)
                    partB = small_pool.tile([P, 1], F32, tag="partB")
                    nc.vector.tensor_reduce(partB, f3, axis=X, op=MIN)
                    nc.vector.tensor_tensor(dist_sb[:, qt:qt + 1], partA, partB, op=MIN)

            # ---- epilogue: transpose [128, 64] -> [64, 128], DMA out ----
            with tc.tile_pool(name="ep_psum", bufs=1, space="PSUM") as ep_psum, \
                 tc.tile_pool(name="ep_sbuf", bufs=1) as ep_sbuf:
                pst = ep_psum.tile([NQT, P], F32)
                nc.tensor.transpose(pst, dist_sb, identity_f32)
                osb = ep_sbuf.tile([NQT, P], F32)
                # true min squared distances are >= 0; the expansion formula
                # can go slightly negative for near-duplicate points
                nc.vector.tensor_scalar_max(osb, pst, 0.0)
                nc.sync.dma_start(out=out_dram.rearrange("(a b) -> a b", b=P), in_=osb)

    nc.compile()
    return nc


_NC_CACHE = None


def _get_program():
    global _NC_CACHE
    if _NC_CACHE is None:
        _NC_CACHE = build_program()
    return _NC_CACHE


def kernel(xyz1: np.ndarray, xyz2: np.ndarray):
    xyz1 = np.ascontiguousarray(np.asarray(xyz1, dtype=np.float32))
    xyz2 = np.ascontiguousarray(np.asarray(xyz2, dtype=np.float32))
    nc = _get_program()
    in_maps = []
    for b in range(B):
        aq1 = _build_aug_np(xyz1[b], True)
        ar2 = _build_aug_np(xyz2[b], False)
        aq2 = _build_aug_np(xyz2[b], True)
        ar1 = _build_aug_np(xyz1[b], False)
        in_maps.append({"aq": aq1, "ar": ar2})   # dist1[b]
        in_maps.append({"aq": aq2, "ar": ar1})   # dist2[b]
    res = run_bass_kernel_spmd(nc, in_maps, core_ids=list(range(2 * B)))
    dist1 = np.stack([np.asarray(res.results[2 * b]["dist"]) for b in range(B)])
    dist2 = np.stack([np.asarray(res.results[2 * b + 1]["dist"]) for b in range(B)])
    return dist1, dist2

